# revision 1
# baseline (speedup 1.0000x reference)
"""Trainium2 Bass kernel for nn_FIB_RNN (GRU encoder + autoregressive
sampling decoder with DenseVariational head).

Contract: kernel(**inputs) takes the FULL unsharded inputs (numpy arrays,
keys as in reference.setup_inputs()) and returns the FULL output
[B, GAMMA, 2] float32.

Strategy: pure data parallelism over the batch dim across 8 NeuronCores
(1024 batch rows per core).  Within a core the GRU state is kept
feature-major [U=128 partitions, batch free] so the recurrent matmul is
lhsT=R_gate[128,128] @ rhs=h[128,512] -> PSUM, and the scalar sequence
input enters as a K=1 outer-product matmul accumulated into the same
PSUM bank.  The tiny DenseVariational weights are sampled on the host
(deterministic given dv_eps) and folded into per-step [128,2] matmuls.

Activation-table note: this toolchain has no softplus PWP table, so
softplus(x) = -Ln(sigmoid(-x)) with the minus sign folded into the
downstream affine consumers.  Gates use the sigmoid set directly; the
decoder alternates sigmoid_and_others <-> natural_log (one table-load
pair per decoder step, enforced by a single full-width Ln that depends
on both batch-chunks' sigmoid outputs).

The decoder output is accumulated feature-major in DRAM ([56, 1024] per
core) and transposed on the host.
"""

import os
import sys
from contextlib import ExitStack

import numpy as np

for _p in ("/opt/trn_rl_repo", "/root/.axon_site/_ro/trn_rl_repo"):
    if os.path.isdir(_p) and _p not in sys.path:
        sys.path.insert(0, _p)

import concourse.bass as bass
import concourse.tile as tile
from concourse import bacc, mybir
from concourse.bass_utils import run_bass_kernel_spmd
from concourse.dve_ops import AFFINE_MUL_REDUCE

F32 = mybir.dt.float32
AF = mybir.ActivationFunctionType
ALU = mybir.AluOpType

U = 128                    # rnn units
T_ENC = 48                 # encoder steps
GAMMA = 28                 # decoder outputs (27 sampled feedback steps)
N_CORES = 8
B_FULL = 8192
BC = B_FULL // N_CORES     # 1024 batch rows per core
CW = 512                   # chunk width (PSUM bank = 512 fp32)
NCH = BC // CW             # 2 chunks per core
C_SP = float(np.log(np.expm1(1.0)))  # softplus^-1(1.0)
Q_SCALE = 0.02
OP_SCALE = 0.05

# matmul operand dtype.  float32r (e8m11-rounded fp32) streams 1 col/cycle
# vs 4 cycles/col for full fp32; bfloat16 streams 2 cols/cycle and gets
# fast weight loads.  All matmul operands (R, K, WK, h, x, y) are declared
# in this dtype end-to-end; constants are pre-rounded on the host.
_MM_MODE = os.environ.get("KERNEL_MM_DT", "f32r")
RD = {"f32r": mybir.dt.float32r, "bf16": mybir.dt.bfloat16, "f32": F32}[_MM_MODE]
RD16 = mybir.dt.bfloat16 if _MM_MODE == "bf16" else F32

_CACHE = {}


def _round_fp32r(a):
    """Round/cast fp32 array to the matmul operand dtype's grid."""
    a = np.ascontiguousarray(a, np.float32)
    if _MM_MODE == "f32":
        return a
    if _MM_MODE == "bf16":
        import ml_dtypes
        return np.ascontiguousarray(a.astype(ml_dtypes.bfloat16))
    bits = a.view(np.uint32)
    out = ((bits.astype(np.uint64) + 0x800) & 0xFFFFF000).astype(np.uint32)
    return out.view(np.float32)


def _build_program(with_b1h):
    """Build + schedule the single-core Bass program (shared by all 8
    cores; per-core data differs only through the input tensors).
    with_b1h: emit the extra recurrent-bias add for the h-gate (only
    needed when gru_bias[1, 2U:3U] is nonzero)."""
    nc = bacc.Bacc("TRN2", target_bir_lowering=False, debug=False)

    x_seq = nc.dram_tensor("x_seq", [T_ENC, BC], RD, kind="ExternalInput").ap()
    eps_seq = nc.dram_tensor("eps_seq", [GAMMA - 1, BC], F32, kind="ExternalInput").ap()
    r_w = nc.dram_tensor("r_w", [U, 3 * U], RD, kind="ExternalInput").ap()
    k_w = nc.dram_tensor("k_w", [1, 3 * U], RD, kind="ExternalInput").ap()
    k_col = nc.dram_tensor("k_col", [U, 3], F32, kind="ExternalInput").ap()
    wk = nc.dram_tensor("wk", [U, 2 * GAMMA], RD, kind="ExternalInput").ap()
    wb0 = nc.dram_tensor("wb0", [1, GAMMA], F32, kind="ExternalInput").ap()
    cb1 = nc.dram_tensor("cb1", [1, GAMMA], F32, kind="ExternalInput").ap()
    gb = nc.dram_tensor("gb", [U, 4], F32, kind="ExternalInput").ap()
    h0_z = nc.dram_tensor("h0_z", [U, BC], RD, kind="ExternalInput").ap()
    out_fm = nc.dram_tensor("out_fm", [2 * GAMMA, BC], F32, kind="ExternalOutput").ap()

    with tile.TileContext(nc) as tc, ExitStack() as es:
        consts = es.enter_context(tc.tile_pool(name="consts", bufs=1))
        R = consts.tile([U, 3 * U], RD)
        K = consts.tile([1, 3 * U], RD)
        KC = consts.tile([U, 3], F32)
        WK = consts.tile([U, 2 * GAMMA], RD)
        WB0 = consts.tile([1, GAMMA], F32)
        CB1 = consts.tile([1, GAMMA], F32)
        GB = consts.tile([U, 4], F32)
        SCB = consts.tile([1, 1], F32)
        nc.vector.memset(SCB[:], 1e-5)
        nc.sync.dma_start(R[:], r_w[:])
        nc.sync.dma_start(K[:], k_w[:])
        nc.sync.dma_start(KC[:], k_col[:])
        nc.sync.dma_start(WK[:], wk[:])
        nc.sync.dma_start(WB0[:], wb0[:])
        nc.sync.dma_start(CB1[:], cb1[:])
        nc.sync.dma_start(GB[:], gb[:])

        hpool = es.enter_context(tc.tile_pool(name="h", bufs=4))
        gates = es.enter_context(tc.tile_pool(name="gates", bufs=3))
        samp = es.enter_context(tc.tile_pool(name="samp", bufs=2))
        stage = es.enter_context(tc.tile_pool(name="stage", bufs=5))
        ps_g = es.enter_context(tc.tile_pool(name="psg", bufs=int(os.environ.get("KERNEL_PS_BUFS", "8")), space="PSUM"))

        h = []
        for c in range(NCH):
            hc = hpool.tile([U, CW], RD, tag=f"h{c}")
            nc.sync.dma_start(hc[:], h0_z[:, bass.ts(c, CW)])
            h.append(hc)

        def gru_step(xb, c, x_row=None):
            """One GRU step for chunk c.  Either xb: [128, BC] broadcast tile
            (encoder; xb[0:1, chunk] doubles as the K=1 matmul rhs) or
            x_row: [1, BC] tile (decoder; x*K_h goes through a PSUM bank)."""
            hc = h[c]
            if x_row is None:
                x_row = xb[0:1, :]
            x_row = x_row[0:1, bass.ts(c, CW)]
            z3 = bass.ts(0, U)  # gate column ranges in R/K
            r3 = bass.ts(1, U)
            psr = ps_g.tile([U, CW], F32, tag="ps")
            psh = ps_g.tile([U, CW], F32, tag="ps")
            psz = ps_g.tile([U, CW], F32, tag="ps")
            psx = None
            if xb is not None:
                # encoder: x is prefetched -- stream K@x first so the
                # post-h2 chain only waits for the R@h matmul.
                nc.tensor.matmul(psr[:], K[:, r3], x_row, start=True, stop=False)
                nc.tensor.matmul(psr[:], R[:, r3], hc[:], start=False, stop=True)
                nc.tensor.matmul(psh[:], R[:, bass.ts(2, U)], hc[:],
                                 start=True, stop=True)
                nc.tensor.matmul(psz[:], K[:, z3], x_row, start=True, stop=False)
                nc.tensor.matmul(psz[:], R[:, z3], hc[:], start=False, stop=True)
            else:
                # decoder: y arrives late -- issue every R@h matmul first
                # (they only need h2), then the K@y matmuls, so the in-order
                # PE isn't blocked behind the y dependency.
                psx = ps_g.tile([U, CW], F32, tag="ps")
                nc.tensor.matmul(psr[:], R[:, r3], hc[:], start=True, stop=False)
                nc.tensor.matmul(psh[:], R[:, bass.ts(2, U)], hc[:],
                                 start=True, stop=True)
                nc.tensor.matmul(psz[:], R[:, z3], hc[:], start=True, stop=False)
                nc.tensor.matmul(psr[:], K[:, r3], x_row, start=False, stop=True)
                nc.tensor.matmul(psx[:], K[:, bass.ts(2, U)], x_row,
                                 start=True, stop=True)
                nc.tensor.matmul(psz[:], K[:, z3], x_row, start=False, stop=True)
            # r = sigmoid(rho + br)                     (GB1 = br)
            r_ = gates.tile([U, CW], RD16, tag=f"r_{c}")
            nc.scalar.activation(r_[:], psr[:], AF.Sigmoid, bias=GB[:, 1:2],
                                 scale=1.0)
            # u1 = 1-z = sigmoid(-(zeta + bz))          (GB0 = -bz)
            u1 = gates.tile([U, CW], RD16, tag=f"u1_{c}")
            nc.scalar.activation(u1[:], psz[:], AF.Sigmoid, bias=GB[:, 0:1],
                                 scale=-1.0)
            # t = r * (hh_rec + b1h)
            hrec = psh
            if with_b1h:
                hb = gates.tile([U, CW], F32, tag=f"hb_{c}")
                nc.vector.tensor_scalar(
                    hb[:], psh[:], GB[:, 3:4], None, op0=ALU.add
                )
                hrec = hb
            tt = gates.tile([U, CW], F32, tag=f"t_{c}")
            nc.vector.tensor_mul(tt[:], r_[:], hrec[:])
            uu = gates.tile([U, CW], F32, tag=f"u_{c}")
            if xb is not None:
                # u = t + x*K_h  (x broadcast tile * per-partition K_h column)
                nc.vector.scalar_tensor_tensor(
                    uu[:], xb[:, bass.ts(c, CW)], KC[:, 2:3], tt[:],
                    op0=ALU.mult, op1=ALU.add,
                )
            else:
                nc.vector.tensor_add(uu[:], tt[:], psx[:])
            hh = gates.tile([U, CW], RD16, tag=f"hh_{c}")
            nc.scalar.activation(hh[:], uu[:], AF.Tanh, bias=GB[:, 2:3], scale=1.0)
            # h' = h + (1-z)*(hh - h): three consecutive DVE ops (no
            # cross-engine hops, and GpSimd stays off the shared SBUF port)
            d = gates.tile([U, CW], RD16, tag=f"d_{c}")
            nc.vector.tensor_sub(d[:], hh[:], hc[:])
            e = gates.tile([U, CW], RD16, tag=f"e_{c}")
            nc.vector.tensor_mul(e[:], u1[:], d[:])
            h2 = hpool.tile([U, CW], RD, tag=f"h{c}")
            nc.vector.tensor_add(h2[:], hc[:], e[:])
            h[c] = h2

        def dense_var(t):
            """DenseVariational head for step t: writes out_fm rows 2t/2t+1.
            Returns (locs per chunk, sp [1,BC]) for sampling."""
            locs = []
            w = samp.tile([1, BC], F32, tag="w")
            for c in range(NCH):
                hc = h[c]
                cs = bass.ts(c, CW)
                psl = ps_g.tile([1, CW], F32, tag="ps")
                nc.tensor.matmul(
                    psl[:], WK[:, 2 * t : 2 * t + 1], hc[:],
                    start=True, stop=True,
                )
                pss = ps_g.tile([1, CW], F32, tag="ps")
                nc.tensor.matmul(
                    pss[:], WK[:, 2 * t + 1 : 2 * t + 2], hc[:],
                    start=True, stop=True,
                )
                # g = sigmoid(-(s + C + wb1))   [sigmoid set; CB1 = -(C+wb1)]
                nc.scalar.activation(
                    w[0:1, cs], pss[:], AF.Sigmoid,
                    bias=CB1[0:1, t : t + 1], scale=-1.0,
                )
                # loc = h@W0 + wb0
                loc = samp.tile([1, CW], F32, tag=f"loc_{c}")
                nc.vector.tensor_scalar(
                    loc[:], psl[:], WB0[0:1, t : t + 1], None, op0=ALU.add
                )
                locs.append(loc)
                nc.sync.dma_start(out_fm[2 * t : 2 * t + 1, cs], loc[:])
            # softplus = -ln(g): ONE full-width Ln so it depends on both
            # chunks' sigmoids -> exactly one table switch per step.  The
            # minus sign is folded into the sc/m consumers.
            sp = samp.tile([1, BC], F32, tag="sp")
            nc.scalar.activation(sp[:], w[:], AF.Ln, bias=0.0, scale=1.0)
            for c in range(NCH):
                cs = bass.ts(c, CW)
                # output scale row: sc = 1e-5 + 0.05*sp
                sc = samp.tile([1, CW], F32, tag=f"sc_{c}")
                nc.scalar.activation(
                    sc[:], sp[0:1, cs], AF.Identity, bias=SCB[0:1, 0:1],
                    scale=-OP_SCALE,
                )
                nc.sync.dma_start(out_fm[2 * t + 1 : 2 * t + 2, cs], sc[:])
            return locs, sp

        def sample(t, locs, sp):
            """y = loc + (1e-5 + 0.05*sp) * eps_t; returns y [1, BC] tile."""
            ep = stage.tile([1, BC], F32, tag="eps")
            nc.sync.dma_start(ep[:], eps_seq[t : t + 1, :])
            y = samp.tile([1, BC], RD, tag="y")
            for c in range(NCH):
                cs = bass.ts(c, CW)
                m = samp.tile([1, CW], F32, tag=f"m_{c}")
                nc.vector._custom_dve(
                    AFFINE_MUL_REDUCE, out=m[:], in0=sp[0:1, cs],
                    in1=ep[0:1, cs], s0=-OP_SCALE, s1=1e-5,
                )
                nc.vector.tensor_add(y[0:1, cs], m[:], locs[c][:])
            return y

        # ---- encoder: 48 GRU steps over the input sequence ----
        for t in range(T_ENC):
            xb = stage.tile([U, BC], RD, tag="xb")
            nc.sync.dma_start(xb[:], x_seq[t : t + 1, :].partition_broadcast(U))
            for c in range(NCH):
                gru_step(xb, c)

        # ---- decoder: dense head + 27 sampled feedback GRU steps ----
        locs, sp = dense_var(0)
        for t in range(1, GAMMA):
            y = sample(t - 1, locs, sp)
            for c in range(NCH):
                gru_step(None, c, x_row=y[:])
            locs, sp = dense_var(t)

    nc.compile()
    return nc


def _host_prep(inputs, gru_kernel, gru_rec_kernel, gru_bias, dv_loc, dv_rho,
               dv_eps, samp_eps):
    """Host-side input preprocessing -> per-core input maps."""
    inputs = np.asarray(inputs, np.float32)
    B = inputs.shape[0]
    assert B == B_FULL, f"kernel compiled for B={B_FULL}, got {B}"
    xT = _round_fp32r(inputs[:, :T_ENC, 0].T)                  # [48, B]
    epsT = np.ascontiguousarray(np.asarray(samp_eps, np.float32)[:, :, 0])  # [27, B]

    gru_bias = np.asarray(gru_bias, np.float32)
    b0, b1 = gru_bias[0], gru_bias[1]
    gb = np.zeros((U, 4), np.float32)
    gb[:, 0] = -(b0[0:U] + b1[0:U])
    gb[:, 1] = b0[U : 2 * U] + b1[U : 2 * U]
    gb[:, 2] = b0[2 * U : 3 * U]
    gb[:, 3] = b1[2 * U : 3 * U]

    dv_loc = np.asarray(dv_loc, np.float32)
    dv_rho = np.asarray(dv_rho, np.float32)
    dv_eps = np.asarray(dv_eps, np.float32)
    scale_q = np.float32(1e-5) + np.float32(Q_SCALE) * np.logaddexp(
        np.float32(C_SP) + dv_rho, np.float32(0.0), dtype=np.float32
    )
    w_all = dv_loc[None, :] + scale_q[None, :] * dv_eps        # [28, 258]
    wk = np.ascontiguousarray(
        w_all[:, : 2 * U].reshape(GAMMA, U, 2).transpose(1, 0, 2).reshape(U, 2 * GAMMA)
    )
    wb0 = np.ascontiguousarray(w_all[:, 2 * U][None, :])       # [1, 28]
    cb1 = np.ascontiguousarray(
        (-(np.float32(C_SP) + w_all[:, 2 * U + 1]))[None, :]
    )  # [1, 28], negated: softplus comes via -ln(sigmoid(-x))

    shared = {
        "r_w": _round_fp32r(gru_rec_kernel),
        "k_w": _round_fp32r(gru_kernel),
        "k_col": np.ascontiguousarray(
            np.asarray(gru_kernel, np.float32).reshape(3, U).T
        ),
        "wk": _round_fp32r(wk),
        "wb0": wb0.astype(np.float32),
        "cb1": cb1.astype(np.float32),
        "gb": gb,
        "h0_z": _round_fp32r(np.zeros((U, BC), np.float32)),
    }
    in_maps = []
    for c in range(N_CORES):
        sl = slice(c * BC, (c + 1) * BC)
        in_maps.append(
            dict(
                shared,
                x_seq=np.ascontiguousarray(xT[:, sl]),  # pre-rounded
                eps_seq=np.ascontiguousarray(epsT[:, sl]),
            )
        )
    return in_maps, bool(np.any(gb[:, 3] != 0.0))


def _get_nc(with_b1h=False):
    key = ("nc", with_b1h)
    if key not in _CACHE:
        _CACHE[key] = _build_program(with_b1h)
    return _CACHE[key]


def run(inputs_dict, trace=False, trace_kwargs=None):
    in_maps, with_b1h = _host_prep(**inputs_dict)
    nc = _get_nc(with_b1h)
    res = run_bass_kernel_spmd(
        nc, in_maps, list(range(N_CORES)), trace=trace,
        **(trace_kwargs or {}),
    )
    _CACHE["last_results"] = res
    out = np.empty((B_FULL, GAMMA, 2), np.float32)
    for c in range(N_CORES):
        fm = res.results[c]["out_fm"]                          # [56, 1024]
        out[c * BC : (c + 1) * BC] = fm.reshape(GAMMA, 2, BC).transpose(2, 0, 1)
    return out


def kernel(**inputs):
    return run(inputs, trace=bool(os.environ.get("KERNEL_TRACE")))



# revision 10
# speedup vs baseline: 538.9078x; 538.9078x over previous
"""Trainium2 Bass kernel for nn_FIB_RNN (GRU encoder + autoregressive
sampling decoder with DenseVariational head).

Contract: kernel(**inputs) takes the FULL unsharded inputs (numpy arrays,
keys as in reference.setup_inputs()) and returns the FULL output
[B, GAMMA, 2] float32.

Strategy: pure data parallelism over the batch dim across 8 NeuronCores
(1024 batch rows per core), feature-major GRU state [U=128, batch].

v2 changes over the 872us baseline:
 - r and (1-z) gates share ONE sigmoid ACT over a 2-bank [128,1024]
   PSUM tile: host negates the z-gate weight columns so u1 = sigmoid(+)
   and reorders gates to {r, -z, h}.
 - gate biases ride a ones-row in an augmented K=2 input matmul
   (zero extra PE cycles), so the merged sigmoid needs no bias tensor.
 - decoder softplus via Exp then Ln(x+1) (both in the
   natural_log_exp_and_others table set): the natural-log table load no
   longer waits on the head matmul, and the raw exp tile E is DMA'd out
   so the host computes the output scale channel as 1e-5+0.05*log1p(E)
   exactly.  loc is reconstructed on host as y - scale*eps from the
   DMA'd feedback sample y (exact up to the f32r rounding of y), which
   kills the per-step loc/scale Identity ACTs and their DMAs.
 - the sample math is fused: AFFINE_MUL_REDUCE gives
   m = (0.05*sp + 1e-5)*eps in one DVE op, scalar_tensor_tensor gives
   y = (psl + wb0) + m in one more.
 - PSUM plan (8 banks exactly): psrz [128,1024] bufs=2 (4 banks) +
   pshx [128,512] bufs=2 shared by psh/psx (2) + head [1,1024] bufs=1
   ring shared by psv/psl (2).
 - optional (KERNEL_GPS=1): chunk-1's h-update chain (d, e, h2) runs on
   the idle GpSimd engine to cut the Vector-engine bottleneck.
"""

import os
import sys
from contextlib import ExitStack

import numpy as np

for _p in ("/opt/trn_rl_repo", "/root/.axon_site/_ro/trn_rl_repo"):
    if os.path.isdir(_p) and _p not in sys.path:
        sys.path.insert(0, _p)

import concourse.bass as bass
import concourse.tile as tile
from concourse import bacc, mybir
from concourse.bass_utils import run_bass_kernel_spmd

F32 = mybir.dt.float32
AF = mybir.ActivationFunctionType
ALU = mybir.AluOpType
RD = mybir.dt.float32r

U = 128                    # rnn units
T_ENC = 48                 # encoder steps
GAMMA = 28                 # decoder outputs (27 sampled feedback steps)
N_CORES = 8
B_FULL = 8192
BC = B_FULL // N_CORES     # 1024 batch rows per core
CW = 512                   # chunk width (PSUM bank = 512 fp32)
NCH = BC // CW             # 2 chunks per core
C_SP = float(np.log(np.expm1(1.0)))  # softplus^-1(1.0)
Q_SCALE = 0.02
OP_SCALE = 0.05

_GPS = os.environ.get("KERNEL_GPS", "1") == "1"

_CACHE = {}


def _round_fp32r(a):
    """Round fp32 array to the f32r (e8m11) grid so CoreSim matches HW."""
    a = np.ascontiguousarray(a, np.float32)
    bits = a.view(np.uint32)
    out = ((bits.astype(np.uint64) + 0x800) & 0xFFFFF000).astype(np.uint32)
    return out.view(np.float32)


def _build_program(with_b1h):
    """Build + schedule the single-core Bass program (shared by all 8
    cores; per-core data differs only through the input tensors)."""
    nc = bacc.Bacc("TRN2", target_bir_lowering=False, debug=False)

    # DRAM tensors.  Gate order everywhere is {r, -z, h} (z negated so
    # u1 = 1-z = sigmoid(+psum) merges with r into one ACT).
    x2_seq = nc.dram_tensor("x2_seq", [T_ENC, 2, BC], RD, kind="ExternalInput").ap()
    xb_seq = nc.dram_tensor("xb_seq", [T_ENC, BC], RD, kind="ExternalInput").ap()
    eps_seq = nc.dram_tensor("eps_seq", [GAMMA - 1, BC], F32, kind="ExternalInput").ap()
    r_w = nc.dram_tensor("r_w", [U, 3 * U], RD, kind="ExternalInput").ap()
    k2_w = nc.dram_tensor("k2_w", [2, 3 * U], RD, kind="ExternalInput").ap()
    k_col = nc.dram_tensor("k_col", [U, 1], F32, kind="ExternalInput").ap()
    wk = nc.dram_tensor("wk", [U, 2 * GAMMA], RD, kind="ExternalInput").ap()
    wb0 = nc.dram_tensor("wb0", [1, GAMMA], F32, kind="ExternalInput").ap()
    cvb = nc.dram_tensor("cvb", [1, GAMMA], F32, kind="ExternalInput").ap()
    gb = nc.dram_tensor("gb", [U, 2], F32, kind="ExternalInput").ap()
    ones_row = nc.dram_tensor("ones_row", [1, BC], RD, kind="ExternalInput").ap()
    h0_z = nc.dram_tensor("h0_z", [U, BC], RD, kind="ExternalInput").ap()
    out_E = nc.dram_tensor("out_E", [GAMMA, BC], F32, kind="ExternalOutput").ap()
    out_y = nc.dram_tensor("out_y", [GAMMA - 1, BC], RD, kind="ExternalOutput").ap()
    out_l27 = nc.dram_tensor("out_l27", [1, BC], F32, kind="ExternalOutput").ap()

    with tile.TileContext(nc) as tc, ExitStack() as es:
        consts = es.enter_context(tc.tile_pool(name="consts", bufs=1))
        R = consts.tile([U, 3 * U], RD)
        K2 = consts.tile([2, 3 * U], RD)
        KC = consts.tile([U, 1], F32)
        WK = consts.tile([U, 2 * GAMMA], RD)
        WB0 = consts.tile([1, GAMMA], F32)
        CVB = consts.tile([1, GAMMA], F32)
        GB = consts.tile([U, 2], F32)
        Y2 = consts.tile([2, BC], RD, name="y2")   # row0 = y, row1 = ones
        ACC = consts.tile([1, 1], F32)             # AMR accum scratch
        nc.sync.dma_start(R[:], r_w[:])
        nc.sync.dma_start(K2[:], k2_w[:])
        nc.sync.dma_start(KC[:], k_col[:])
        nc.sync.dma_start(WK[:], wk[:])
        nc.sync.dma_start(WB0[:], wb0[:])
        nc.sync.dma_start(CVB[:], cvb[:])
        nc.sync.dma_start(GB[:], gb[:])
        nc.sync.dma_start(Y2[1:2, :], ones_row[:])  # row0 overwritten per step

        hpool = es.enter_context(tc.tile_pool(name="h", bufs=3))
        gates = es.enter_context(tc.tile_pool(name="gates", bufs=2))
        samp = es.enter_context(tc.tile_pool(name="samp", bufs=2))
        stage = es.enter_context(tc.tile_pool(name="stage", bufs=2))
        ps_g = es.enter_context(tc.tile_pool(name="psg", bufs=2, space="PSUM"))

        h = []
        for c in range(NCH):
            hc = hpool.tile([U, CW], RD, tag=f"h{c}")
            nc.sync.dma_start(hc[:], h0_z[:, bass.ts(c, CW)])
            h.append(hc)

        rs = bass.ts(0, U)       # gate column ranges in R/K2: {r, -z, h}
        zs = bass.ts(1, U)
        hs = bass.ts(2, U)

        def gru_step(c, xb=None, x2=None, y2=None):
            """One GRU step for chunk c.  Encoder: xb [U,BC] broadcast
            tile + x2 [2,BC] augmented row-pair.  Decoder: y2 [2,BC]
            (row0=y, row1=ones)."""
            hc = h[c]
            cs = bass.ts(c, CW)
            in2 = x2 if y2 is None else y2
            psrz = ps_g.tile([U, 2 * CW], F32, tag="psrz", bufs=2)
            psh = ps_g.tile([U, CW], F32, tag="pshx", bufs=2)
            psx = None
            if y2 is None:
                # encoder: x is prefetched -- stream K@x first so the
                # post-h2 chain only waits for the R@h matmul.
                nc.tensor.matmul(psrz[:, 0:CW], K2[:, rs], in2[:, cs],
                                 start=True, stop=False)
                nc.tensor.matmul(psrz[:, 0:CW], R[:, rs], hc[:],
                                 start=False, stop=True)
                nc.tensor.matmul(psrz[:, CW:], K2[:, zs], in2[:, cs],
                                 start=True, stop=False)
                nc.tensor.matmul(psrz[:, CW:], R[:, zs], hc[:],
                                 start=False, stop=True)
                nc.tensor.matmul(psh[:], R[:, hs], hc[:], start=True, stop=True)
            else:
                # decoder: y arrives late -- issue every R@h matmul
                # first, then the K@y matmuls.
                psx = ps_g.tile([U, CW], F32, tag="pshx", bufs=2)
                nc.tensor.matmul(psrz[:, 0:CW], R[:, rs], hc[:],
                                 start=True, stop=False)
                nc.tensor.matmul(psrz[:, CW:], R[:, zs], hc[:],
                                 start=True, stop=False)
                nc.tensor.matmul(psh[:], R[:, hs], hc[:], start=True, stop=True)
                nc.tensor.matmul(psrz[:, 0:CW], K2[:, rs], in2[:, cs],
                                 start=False, stop=True)
                nc.tensor.matmul(psrz[:, CW:], K2[:, zs], in2[:, cs],
                                 start=False, stop=True)
                nc.tensor.matmul(psx[:], K2[0:1, hs], in2[0:1, cs],
                                 start=True, stop=True)
            # one sigmoid for both gates: G = [r | u1]
            G = gates.tile([U, 2 * CW], F32, tag=f"G_{c}")
            nc.scalar.activation(G[:], psrz[:], AF.Sigmoid, bias=0.0, scale=1.0)
            hrec = psh
            if with_b1h:
                hb = gates.tile([U, CW], F32, tag=f"hb_{c}")
                nc.vector.tensor_scalar(hb[:], psh[:], GB[:, 1:2], None,
                                        op0=ALU.add)
                hrec = hb
            tt = gates.tile([U, CW], F32, tag=f"t_{c}")
            nc.vector.tensor_mul(tt[:], G[:, 0:CW], hrec[:])
            uu = gates.tile([U, CW], F32, tag=f"u_{c}")
            if y2 is None:
                # u = t + x*K_h  (x broadcast tile * per-partition K_h)
                nc.vector.scalar_tensor_tensor(
                    uu[:], xb[:, cs], KC[:, 0:1], tt[:],
                    op0=ALU.mult, op1=ALU.add,
                )
            else:
                nc.vector.tensor_add(uu[:], tt[:], psx[:])
            hh = gates.tile([U, CW], F32, tag=f"hh_{c}")
            nc.scalar.activation(hh[:], uu[:], AF.Tanh, bias=GB[:, 0:1],
                                 scale=1.0)
            # h' = h + (1-z)*(hh - h)
            eng = nc.gpsimd if (_GPS and c == 1) else nc.vector
            d = gates.tile([U, CW], F32, tag=f"d_{c}")
            eng.tensor_sub(d[:], hh[:], hc[:])
            e = gates.tile([U, CW], F32, tag=f"e_{c}")
            eng.tensor_mul(e[:], G[:, CW:], d[:])
            h2 = hpool.tile([U, CW], RD, tag=f"h{c}")
            eng.tensor_add(h2[:], hc[:], e[:])
            h[c] = h2

        def dense_head(t):
            """DenseVariational head for step t.  Returns (psl, SP):
            psl [1,BC] PSUM (raw h@w0), SP [1,BC] SBUF (softplus row).
            DMAs the raw exp row E for host post-processing."""
            psv = ps_g.tile([1, 2 * CW], F32, tag="pshead", bufs=1)
            for c in range(NCH):
                nc.tensor.matmul(psv[0:1, bass.ts(c, CW)],
                                 WK[:, 2 * t + 1 : 2 * t + 2],
                                 h[c][:], start=True, stop=True)
            # E = exp(v + C + wb1_t);  host: scale = 1e-5+0.05*log1p(E)
            E = samp.tile([1, BC], F32, tag="E")
            nc.scalar.activation(E[:], psv[:], AF.Exp,
                                 bias=CVB[0:1, t : t + 1], scale=1.0)
            nc.sync.dma_start(out_E[t : t + 1, :], E[:])
            psl = ps_g.tile([1, 2 * CW], F32, tag="pshead", bufs=1)
            for c in range(NCH):
                nc.tensor.matmul(psl[0:1, bass.ts(c, CW)],
                                 WK[:, 2 * t : 2 * t + 1],
                                 h[c][:], start=True, stop=True)
            if t == GAMMA - 1:
                # last step: only the outputs are needed
                l27 = samp.tile([1, BC], F32, tag="l27")
                nc.vector.tensor_scalar(l27[:], psl[:], WB0[0:1, t : t + 1],
                                        None, op0=ALU.add)
                nc.sync.dma_start(out_l27[0:1, :], l27[:])
                return None, None
            # sp = ln(1 + E)
            SP = samp.tile([1, BC], F32, tag="SP")
            nc.scalar.activation(SP[:], E[:], AF.Ln, bias=1.0, scale=1.0)
            return psl, SP

        def sample(t, psl, SP):
            """y = (psl + wb0_t) + (0.05*sp + 1e-5)*eps_t, into Y2 row 0."""
            ep = stage.tile([1, BC], F32, tag="eps", bufs=3)
            nc.sync.dma_start(ep[:], eps_seq[t : t + 1, :])
            M = samp.tile([1, BC], F32, tag="M")
            nc.vector.affine_mul_reduce(
                M[:], ACC[0:1, :], SP[:], ep[:], OP_SCALE, 1e-5,
            )
            nc.vector.scalar_tensor_tensor(
                Y2[0:1, :], psl[:], WB0[0:1, t : t + 1], M[:],
                op0=ALU.add, op1=ALU.add,
            )
            nc.sync.dma_start(out_y[t : t + 1, :], Y2[0:1, :])

        # ---- encoder: 48 GRU steps over the input sequence ----
        for t in range(T_ENC):
            xb = stage.tile([U, BC], RD, tag="xb")
            nc.sync.dma_start(xb[:], xb_seq[t : t + 1, :].partition_broadcast(U))
            x2 = stage.tile([2, BC], RD, tag="x2")
            nc.sync.dma_start(x2[:], x2_seq[t, :, :])
            for c in range(NCH):
                gru_step(c, xb=xb, x2=x2)

        # ---- decoder: dense head + 27 sampled feedback GRU steps ----
        psl, SP = dense_head(0)
        for t in range(1, GAMMA):
            sample(t - 1, psl, SP)
            for c in range(NCH):
                gru_step(c, y2=Y2)
            psl, SP = dense_head(t)

    nc.compile()
    return nc


def _host_prep(inputs, gru_kernel, gru_rec_kernel, gru_bias, dv_loc, dv_rho,
               dv_eps, samp_eps):
    """Host-side input preprocessing -> per-core input maps."""
    inputs = np.asarray(inputs, np.float32)
    B = inputs.shape[0]
    assert B == B_FULL, f"kernel compiled for B={B_FULL}, got {B}"
    xT = _round_fp32r(inputs[:, :T_ENC, 0].T)                  # [48, B]
    epsT = np.ascontiguousarray(np.asarray(samp_eps, np.float32)[:, :, 0])  # [27, B]

    gru_bias = np.asarray(gru_bias, np.float32)
    b0, b1 = gru_bias[0], gru_bias[1]
    gk = np.asarray(gru_kernel, np.float32)[0]                 # [3U]
    rk = np.asarray(gru_rec_kernel, np.float32)                # [U, 3U]

    # gate reorder {z,r,h} -> {r, -z, h}; z columns negated
    def reorder(m, axis):
        z, r, hh_ = np.split(m, 3, axis=axis)
        return np.concatenate([r, -z, hh_], axis=axis)

    r_w = reorder(rk, 1)                                       # [U, 3U]
    k_row = reorder(gk[None, :], 1)                            # [1, 3U]
    bias_rz = reorder((b0 + b1)[None, :], 1)                   # [1, 3U]
    bias_rz[0, 2 * U :] = 0.0                                  # h-gate bias via ACT
    k2 = np.concatenate([k_row, bias_rz], axis=0)              # [2, 3U]

    gb = np.zeros((U, 2), np.float32)
    gb[:, 0] = b0[2 * U : 3 * U]                               # tanh bias
    gb[:, 1] = b1[2 * U : 3 * U]                               # recurrent h bias

    dv_loc = np.asarray(dv_loc, np.float32)
    dv_rho = np.asarray(dv_rho, np.float32)
    dv_eps = np.asarray(dv_eps, np.float32)
    scale_q = np.float32(1e-5) + np.float32(Q_SCALE) * np.logaddexp(
        np.float32(C_SP) + dv_rho, np.float32(0.0), dtype=np.float32
    )
    w_all = dv_loc[None, :] + scale_q[None, :] * dv_eps        # [28, 258]
    wk = np.ascontiguousarray(
        w_all[:, : 2 * U].reshape(GAMMA, U, 2).transpose(1, 0, 2).reshape(U, 2 * GAMMA)
    )
    wb0 = np.ascontiguousarray(w_all[:, 2 * U][None, :])       # [1, 28]
    cvb = np.ascontiguousarray(
        (np.float32(C_SP) + w_all[:, 2 * U + 1])[None, :]
    )  # [1, 28]: exp bias = C + wb1_t

    x2_seq = np.empty((T_ENC, 2, B_FULL), np.float32)
    x2_seq[:, 0, :] = xT
    x2_seq[:, 1, :] = 1.0

    shared = {
        "r_w": _round_fp32r(r_w),
        "k2_w": _round_fp32r(k2),
        "k_col": np.ascontiguousarray(gk[2 * U :, None]),      # [U,1] K_h col
        "wk": _round_fp32r(wk),
        "wb0": wb0.astype(np.float32),
        "cvb": cvb.astype(np.float32),
        "gb": gb,
        "ones_row": np.ones((1, BC), np.float32),
        "h0_z": np.zeros((U, BC), np.float32),
    }
    in_maps = []
    for c in range(N_CORES):
        sl = slice(c * BC, (c + 1) * BC)
        in_maps.append(
            dict(
                shared,
                xb_seq=np.ascontiguousarray(xT[:, sl]),
                x2_seq=np.ascontiguousarray(x2_seq[:, :, sl]),
                eps_seq=np.ascontiguousarray(epsT[:, sl]),
            )
        )
    return in_maps, bool(np.any(gb[:, 1] != 0.0)), epsT


def _get_nc(with_b1h=False):
    key = ("nc", with_b1h, _GPS)
    if key not in _CACHE:
        _CACHE[key] = _build_program(with_b1h)
    return _CACHE[key]


def _postprocess(res_list, epsT):
    """Assemble [B, GAMMA, 2] from per-core {out_E, out_y, out_l27}."""
    out = np.empty((B_FULL, GAMMA, 2), np.float32)
    for c in range(N_CORES):
        sl = slice(c * BC, (c + 1) * BC)
        E = np.asarray(res_list[c]["out_E"], np.float64)       # [28, BC]
        y = np.asarray(res_list[c]["out_y"], np.float64)       # [27, BC]
        l27 = np.asarray(res_list[c]["out_l27"], np.float64)   # [1, BC]
        scale = 1e-5 + OP_SCALE * np.log1p(E)                  # [28, BC]
        loc = np.empty((GAMMA, BC))
        loc[:-1] = y - scale[:-1] * epsT[:, sl]
        loc[-1] = l27[0]
        out[sl, :, 0] = loc.T
        out[sl, :, 1] = scale.T
    return out


def run(inputs_dict, trace=False, trace_kwargs=None):
    in_maps, with_b1h, epsT = _host_prep(**inputs_dict)
    nc = _get_nc(with_b1h)
    res = run_bass_kernel_spmd(
        nc, in_maps, list(range(N_CORES)), trace=trace,
        **(trace_kwargs or {}),
    )
    _CACHE["last_results"] = res
    return _postprocess(res.results, epsT)


def kernel(**inputs):
    return run(inputs, trace=bool(os.environ.get("KERNEL_TRACE")))


# revision 17
# speedup vs baseline: 697.8911x; 1.2950x over previous
"""Trainium2 Bass kernel for nn_FIB_RNN (GRU encoder + autoregressive
sampling decoder with DenseVariational head).

Contract: kernel(**inputs) takes the FULL unsharded inputs (numpy arrays,
keys as in reference.setup_inputs()) and returns the FULL output
[B, GAMMA, 2] float32.

Strategy: pure data parallelism over the batch dim across 8 NeuronCores
(1024 batch rows per core), feature-major GRU state [U=128, batch].

v2 changes over the 872us baseline:
 - r and (1-z) gates share ONE sigmoid ACT over a 2-bank [128,1024]
   PSUM tile: host negates the z-gate weight columns so u1 = sigmoid(+)
   and reorders gates to {r, -z, h}.
 - gate biases ride a ones-row in an augmented K=2 input matmul
   (zero extra PE cycles), so the merged sigmoid needs no bias tensor.
 - decoder softplus via Exp then Ln(x+1) (both in the
   natural_log_exp_and_others table set): the natural-log table load no
   longer waits on the head matmul, and the raw exp tile E is DMA'd out
   so the host computes the output scale channel as 1e-5+0.05*log1p(E)
   exactly.  loc is reconstructed on host as y - scale*eps from the
   DMA'd feedback sample y (exact up to the f32r rounding of y), which
   kills the per-step loc/scale Identity ACTs and their DMAs.
 - the sample math is fused: AFFINE_MUL_REDUCE gives
   m = (0.05*sp + 1e-5)*eps in one DVE op, scalar_tensor_tensor gives
   y = (psl + wb0) + m in one more.
 - PSUM plan (8 banks exactly): psrz [128,1024] bufs=2 (4 banks) +
   pshx [128,512] bufs=2 shared by psh/psx (2) + head [1,1024] bufs=1
   ring shared by psv/psl (2).
 - optional (KERNEL_GPS=1): chunk-1's h-update chain (d, e, h2) runs on
   the idle GpSimd engine to cut the Vector-engine bottleneck.
"""

import os
import sys
from contextlib import ExitStack

import numpy as np

for _p in ("/opt/trn_rl_repo", "/root/.axon_site/_ro/trn_rl_repo"):
    if os.path.isdir(_p) and _p not in sys.path:
        sys.path.insert(0, _p)

import concourse.bass as bass
import concourse.tile as tile
from concourse import bacc, mybir
from concourse.bass_utils import run_bass_kernel_spmd

F32 = mybir.dt.float32
AF = mybir.ActivationFunctionType
ALU = mybir.AluOpType
# matmul operand dtype: fp16 (e5m10) streams 2 cols/cycle + gets FWL
# fast weight loads; 10-bit mantissa keeps the recurrent rounding error
# ~8x below bf16's.  KERNEL_MM_DT=f32r for the conservative fallback.
_MM_MODE = os.environ.get("KERNEL_MM_DT", "fp16")
RD = {"fp16": mybir.dt.float16, "f32r": mybir.dt.float32r}[_MM_MODE]
# 16-bit dtype for DVE-internal tiles (G, hh, d, e): enables 2x DVE mode
RD16 = mybir.dt.float16 if _MM_MODE == "fp16" else F32

U = 128                    # rnn units
T_ENC = 48                 # encoder steps
GAMMA = 28                 # decoder outputs (27 sampled feedback steps)
N_CORES = 8
B_FULL = 8192
BC = B_FULL // N_CORES     # 1024 batch rows per core
CW = 512                   # chunk width (PSUM bank = 512 fp32)
NCH = BC // CW             # 2 chunks per core
C_SP = float(np.log(np.expm1(1.0)))  # softplus^-1(1.0)
Q_SCALE = 0.02
OP_SCALE = 0.05

_GPS = os.environ.get("KERNEL_GPS", "0") == "1"

_CACHE = {}


def _round_rd(a):
    """Cast fp32 array to the matmul operand dtype's numpy storage."""
    a = np.ascontiguousarray(a, np.float32)
    if _MM_MODE == "fp16":
        return np.ascontiguousarray(a.astype(np.float16))
    bits = a.view(np.uint32)
    out = ((bits.astype(np.uint64) + 0x800) & 0xFFFFF000).astype(np.uint32)
    return out.view(np.float32)


def _pin_act_tables(arch):
    """Hide Exp/Ln from the single-function table sets so the compiler's
    table-load placement resolves both to natural_log_exp_and_others
    (one load covers the decoder's Exp+Ln pair).  Mutates the cached
    dict in place; set positions (= walrus set ids) are unchanged, and
    the real on-device tables still contain the hidden entries, so this
    only steers placement, never correctness."""
    from concourse.hw_specs import get_activation_tables

    tabs = get_activation_tables(arch)
    for name in ("exp_and_others", "exp_and_friends"):
        if name in tabs:
            tabs[name].discard(AF.Exp)
    if "natural_log" in tabs:
        tabs["natural_log"].discard(AF.Ln)


def _build_program(with_b1h):
    """Build + schedule the single-core Bass program (shared by all 8
    cores; per-core data differs only through the input tensors)."""
    nc = bacc.Bacc("TRN2", target_bir_lowering=False, debug=False)
    _pin_act_tables(nc.m.arch)

    # DRAM tensors.  Gate order everywhere is {r, -z, h} (z negated so
    # u1 = 1-z = sigmoid(+psum) merges with r into one ACT).
    x2_seq = nc.dram_tensor("x2_seq", [T_ENC, 2, BC], RD, kind="ExternalInput").ap()
    xb_seq = nc.dram_tensor("xb_seq", [T_ENC, BC], RD, kind="ExternalInput").ap()
    eps_seq = nc.dram_tensor("eps_seq", [GAMMA - 1, BC], F32, kind="ExternalInput").ap()
    r_w = nc.dram_tensor("r_w", [U, 3 * U], RD, kind="ExternalInput").ap()
    k2_w = nc.dram_tensor("k2_w", [2, 3 * U], RD, kind="ExternalInput").ap()
    k_col = nc.dram_tensor("k_col", [U, 1], F32, kind="ExternalInput").ap()
    wk = nc.dram_tensor("wk", [U, 2 * GAMMA], RD, kind="ExternalInput").ap()
    wb0 = nc.dram_tensor("wb0", [1, GAMMA], F32, kind="ExternalInput").ap()
    cvb = nc.dram_tensor("cvb", [1, GAMMA], F32, kind="ExternalInput").ap()
    gb = nc.dram_tensor("gb", [U, 2], F32, kind="ExternalInput").ap()
    ones_row = nc.dram_tensor("ones_row", [1, BC], RD, kind="ExternalInput").ap()
    h0_z = nc.dram_tensor("h0_z", [U, BC], RD, kind="ExternalInput").ap()
    out_E = nc.dram_tensor("out_E", [GAMMA, BC], F32, kind="ExternalOutput").ap()
    out_y = nc.dram_tensor("out_y", [GAMMA - 1, BC], RD, kind="ExternalOutput").ap()
    out_l27 = nc.dram_tensor("out_l27", [1, BC], F32, kind="ExternalOutput").ap()

    with tile.TileContext(nc) as tc, ExitStack() as es:
        consts = es.enter_context(tc.tile_pool(name="consts", bufs=1))
        R = consts.tile([U, 3 * U], RD)
        K2 = consts.tile([2, 3 * U], RD)
        KC = consts.tile([U, 1], F32)
        WK = consts.tile([U, 2 * GAMMA], RD)
        WB0 = consts.tile([1, GAMMA], F32)
        CVB = consts.tile([1, GAMMA], F32)
        GB = consts.tile([U, 2], F32)
        Y2 = consts.tile([2, BC], RD, name="y2")   # row0 = y, row1 = ones
        ACC = consts.tile([1, 1], F32)             # AMR accum scratch
        nc.sync.dma_start(R[:], r_w[:])
        nc.sync.dma_start(K2[:], k2_w[:])
        nc.sync.dma_start(KC[:], k_col[:])
        nc.sync.dma_start(WK[:], wk[:])
        nc.sync.dma_start(WB0[:], wb0[:])
        nc.sync.dma_start(CVB[:], cvb[:])
        nc.sync.dma_start(GB[:], gb[:])
        nc.sync.dma_start(Y2[1:2, :], ones_row[:])  # row0 overwritten per step

        hpool = es.enter_context(tc.tile_pool(name="h", bufs=3))
        gates = es.enter_context(tc.tile_pool(name="gates", bufs=2))
        samp = es.enter_context(tc.tile_pool(name="samp", bufs=2))
        stage = es.enter_context(tc.tile_pool(name="stage", bufs=2))
        ps_g = es.enter_context(tc.tile_pool(name="psg", bufs=2, space="PSUM"))

        h = []
        for c in range(NCH):
            hc = hpool.tile([U, CW], RD, tag=f"h{c}")
            nc.sync.dma_start(hc[:], h0_z[:, bass.ts(c, CW)])
            h.append(hc)

        rs = bass.ts(0, U)       # gate column ranges in R/K2: {r, -z, h}
        zs = bass.ts(1, U)
        hs = bass.ts(2, U)

        def gru_step(c, xb=None, x2=None, y2=None):
            """One GRU step for chunk c.  Encoder: xb [U,BC] broadcast
            tile + x2 [2,BC] augmented row-pair.  Decoder: y2 [2,BC]
            (row0=y, row1=ones)."""
            hc = h[c]
            cs = bass.ts(c, CW)
            in2 = x2 if y2 is None else y2
            psrz = ps_g.tile([U, 2 * CW], F32, tag="psrz", bufs=2)
            psh = ps_g.tile([U, CW], F32, tag="pshx", bufs=2)
            psx = None
            if y2 is None:
                # encoder: x is prefetched -- stream K@x first so the
                # post-h2 chain only waits for the R@h matmul.
                nc.tensor.matmul(psrz[:, 0:CW], K2[:, rs], in2[:, cs],
                                 start=True, stop=False)
                nc.tensor.matmul(psrz[:, 0:CW], R[:, rs], hc[:],
                                 start=False, stop=True)
                nc.tensor.matmul(psrz[:, CW:], K2[:, zs], in2[:, cs],
                                 start=True, stop=False)
                nc.tensor.matmul(psrz[:, CW:], R[:, zs], hc[:],
                                 start=False, stop=True)
                nc.tensor.matmul(psh[:], R[:, hs], hc[:], start=True, stop=True)
            else:
                # decoder: y arrives late -- issue every R@h matmul
                # first, then the K@y matmuls.
                psx = ps_g.tile([U, CW], F32, tag="pshx", bufs=2)
                nc.tensor.matmul(psrz[:, 0:CW], R[:, rs], hc[:],
                                 start=True, stop=False)
                nc.tensor.matmul(psrz[:, CW:], R[:, zs], hc[:],
                                 start=True, stop=False)
                nc.tensor.matmul(psh[:], R[:, hs], hc[:], start=True, stop=True)
                nc.tensor.matmul(psrz[:, 0:CW], K2[:, rs], in2[:, cs],
                                 start=False, stop=True)
                nc.tensor.matmul(psrz[:, CW:], K2[:, zs], in2[:, cs],
                                 start=False, stop=True)
                nc.tensor.matmul(psx[:], K2[0:1, hs], in2[0:1, cs],
                                 start=True, stop=True)
            # one sigmoid for both gates: G = [r | u1]
            G = gates.tile([U, 2 * CW], RD16, tag=f"G_{c}")
            nc.scalar.activation(G[:], psrz[:], AF.Sigmoid, bias=0.0, scale=1.0)
            hrec = psh
            if with_b1h:
                hb = gates.tile([U, CW], F32, tag=f"hb_{c}")
                nc.vector.tensor_scalar(hb[:], psh[:], GB[:, 1:2], None,
                                        op0=ALU.add)
                hrec = hb
            tt = gates.tile([U, CW], F32, tag=f"t_{c}")
            nc.vector.tensor_mul(tt[:], G[:, 0:CW], hrec[:])
            uu = gates.tile([U, CW], F32, tag=f"u_{c}")
            if y2 is None:
                # u = t + x*K_h  (x broadcast tile * per-partition K_h)
                nc.vector.scalar_tensor_tensor(
                    uu[:], xb[:, cs], KC[:, 0:1], tt[:],
                    op0=ALU.mult, op1=ALU.add,
                )
            else:
                nc.vector.tensor_add(uu[:], tt[:], psx[:])
            hh = gates.tile([U, CW], RD16, tag=f"hh_{c}")
            nc.scalar.activation(hh[:], uu[:], AF.Tanh, bias=GB[:, 0:1],
                                 scale=1.0)
            # h' = h + (1-z)*(hh - h)
            eng = nc.gpsimd if (_GPS and c == 1) else nc.vector
            d = gates.tile([U, CW], RD16, tag=f"d_{c}")
            eng.tensor_sub(d[:], hh[:], hc[:])
            e = gates.tile([U, CW], RD16, tag=f"e_{c}")
            eng.tensor_mul(e[:], G[:, CW:], d[:])
            h2 = hpool.tile([U, CW], RD, tag=f"h{c}")
            eng.tensor_add(h2[:], hc[:], e[:])
            h[c] = h2

        def dense_head(t):
            """DenseVariational head for step t.  Returns (psl, SP):
            psl [1,BC] PSUM (raw h@w0), SP [1,BC] SBUF (softplus row).
            DMAs the raw exp row E for host post-processing."""
            psv = ps_g.tile([1, 2 * CW], F32, tag="pshead", bufs=1)
            for c in range(NCH):
                nc.tensor.matmul(psv[0:1, bass.ts(c, CW)],
                                 WK[:, 2 * t + 1 : 2 * t + 2],
                                 h[c][:], start=True, stop=True)
            # E = exp(v + C + wb1_t);  host: scale = 1e-5+0.05*log1p(E)
            E = samp.tile([1, BC], F32, tag="E")
            nc.scalar.activation(E[:], psv[:], AF.Exp,
                                 bias=CVB[0:1, t : t + 1], scale=1.0)
            nc.sync.dma_start(out_E[t : t + 1, :], E[:])
            psl = ps_g.tile([1, 2 * CW], F32, tag="pshead", bufs=1)
            for c in range(NCH):
                nc.tensor.matmul(psl[0:1, bass.ts(c, CW)],
                                 WK[:, 2 * t : 2 * t + 1],
                                 h[c][:], start=True, stop=True)
            if t == GAMMA - 1:
                # last step: only the outputs are needed
                l27 = samp.tile([1, BC], F32, tag="l27")
                nc.vector.tensor_scalar(l27[:], psl[:], WB0[0:1, t : t + 1],
                                        None, op0=ALU.add)
                nc.sync.dma_start(out_l27[0:1, :], l27[:])
                return None, None
            # sp = ln(1 + E)
            SP = samp.tile([1, BC], F32, tag="SP")
            nc.scalar.activation(SP[:], E[:], AF.Ln, bias=1.0, scale=1.0)
            return psl, SP

        def sample(t, psl, SP):
            """y = (psl + wb0_t) + (0.05*sp + 1e-5)*eps_t, into Y2 row 0."""
            ep = stage.tile([1, BC], F32, tag="eps", bufs=3)
            nc.sync.dma_start(ep[:], eps_seq[t : t + 1, :])
            M = samp.tile([1, BC], F32, tag="M")
            nc.vector.affine_mul_reduce(
                M[:], ACC[0:1, :], SP[:], ep[:], OP_SCALE, 1e-5,
            )
            nc.vector.scalar_tensor_tensor(
                Y2[0:1, :], psl[:], WB0[0:1, t : t + 1], M[:],
                op0=ALU.add, op1=ALU.add,
            )
            nc.sync.dma_start(out_y[t : t + 1, :], Y2[0:1, :])

        # ---- encoder: 48 GRU steps over the input sequence ----
        for t in range(T_ENC):
            xb = stage.tile([U, BC], RD, tag="xb")
            nc.sync.dma_start(xb[:], xb_seq[t : t + 1, :].partition_broadcast(U))
            x2 = stage.tile([2, BC], RD, tag="x2")
            nc.sync.dma_start(x2[:], x2_seq[t, :, :])
            for c in range(NCH):
                gru_step(c, xb=xb, x2=x2)

        # ---- decoder: dense head + 27 sampled feedback GRU steps ----
        psl, SP = dense_head(0)
        for t in range(1, GAMMA):
            sample(t - 1, psl, SP)
            for c in range(NCH):
                gru_step(c, y2=Y2)
            psl, SP = dense_head(t)

    nc.compile()
    return nc


def _host_prep(inputs, gru_kernel, gru_rec_kernel, gru_bias, dv_loc, dv_rho,
               dv_eps, samp_eps):
    """Host-side input preprocessing -> per-core input maps."""
    inputs = np.asarray(inputs, np.float32)
    B = inputs.shape[0]
    assert B == B_FULL, f"kernel compiled for B={B_FULL}, got {B}"
    xT = _round_rd(inputs[:, :T_ENC, 0].T)                     # [48, B]
    epsT = np.ascontiguousarray(np.asarray(samp_eps, np.float32)[:, :, 0])  # [27, B]

    gru_bias = np.asarray(gru_bias, np.float32)
    b0, b1 = gru_bias[0], gru_bias[1]
    gk = np.asarray(gru_kernel, np.float32)[0]                 # [3U]
    rk = np.asarray(gru_rec_kernel, np.float32)                # [U, 3U]

    # gate reorder {z,r,h} -> {r, -z, h}; z columns negated
    def reorder(m, axis):
        z, r, hh_ = np.split(m, 3, axis=axis)
        return np.concatenate([r, -z, hh_], axis=axis)

    r_w = reorder(rk, 1)                                       # [U, 3U]
    k_row = reorder(gk[None, :], 1)                            # [1, 3U]
    bias_rz = reorder((b0 + b1)[None, :], 1)                   # [1, 3U]
    bias_rz[0, 2 * U :] = 0.0                                  # h-gate bias via ACT
    k2 = np.concatenate([k_row, bias_rz], axis=0)              # [2, 3U]

    gb = np.zeros((U, 2), np.float32)
    gb[:, 0] = b0[2 * U : 3 * U]                               # tanh bias
    gb[:, 1] = b1[2 * U : 3 * U]                               # recurrent h bias

    dv_loc = np.asarray(dv_loc, np.float32)
    dv_rho = np.asarray(dv_rho, np.float32)
    dv_eps = np.asarray(dv_eps, np.float32)
    scale_q = np.float32(1e-5) + np.float32(Q_SCALE) * np.logaddexp(
        np.float32(C_SP) + dv_rho, np.float32(0.0), dtype=np.float32
    )
    w_all = dv_loc[None, :] + scale_q[None, :] * dv_eps        # [28, 258]
    wk = np.ascontiguousarray(
        w_all[:, : 2 * U].reshape(GAMMA, U, 2).transpose(1, 0, 2).reshape(U, 2 * GAMMA)
    )
    wb0 = np.ascontiguousarray(w_all[:, 2 * U][None, :])       # [1, 28]
    cvb = np.ascontiguousarray(
        (np.float32(C_SP) + w_all[:, 2 * U + 1])[None, :]
    )  # [1, 28]: exp bias = C + wb1_t

    np_rd = np.float16 if _MM_MODE == "fp16" else np.float32
    x2_seq = np.empty((T_ENC, 2, B_FULL), np_rd)
    x2_seq[:, 0, :] = xT
    x2_seq[:, 1, :] = 1.0

    shared = {
        "r_w": _round_rd(r_w),
        "k2_w": _round_rd(k2),
        "k_col": np.ascontiguousarray(gk[2 * U :, None]),      # [U,1] K_h col
        "wk": _round_rd(wk),
        "wb0": wb0.astype(np.float32),
        "cvb": cvb.astype(np.float32),
        "gb": gb,
        "ones_row": np.ones((1, BC), np_rd),
        "h0_z": np.zeros((U, BC), np_rd),
    }
    in_maps = []
    for c in range(N_CORES):
        sl = slice(c * BC, (c + 1) * BC)
        in_maps.append(
            dict(
                shared,
                xb_seq=np.ascontiguousarray(xT[:, sl]),
                x2_seq=np.ascontiguousarray(x2_seq[:, :, sl]),
                eps_seq=np.ascontiguousarray(epsT[:, sl]),
            )
        )
    return in_maps, bool(np.any(gb[:, 1] != 0.0)), epsT


def _get_nc(with_b1h=False):
    key = ("nc", with_b1h, _GPS)
    if key not in _CACHE:
        _CACHE[key] = _build_program(with_b1h)
    return _CACHE[key]


def _postprocess(res_list, epsT):
    """Assemble [B, GAMMA, 2] from per-core {out_E, out_y, out_l27}."""
    out = np.empty((B_FULL, GAMMA, 2), np.float32)
    for c in range(N_CORES):
        sl = slice(c * BC, (c + 1) * BC)
        E = np.asarray(res_list[c]["out_E"], np.float64)       # [28, BC]
        y = np.asarray(res_list[c]["out_y"], np.float64)       # [27, BC]
        l27 = np.asarray(res_list[c]["out_l27"], np.float64)   # [1, BC]
        scale = 1e-5 + OP_SCALE * np.log1p(E)                  # [28, BC]
        loc = np.empty((GAMMA, BC))
        loc[:-1] = y - scale[:-1] * epsT[:, sl]
        loc[-1] = l27[0]
        out[sl, :, 0] = loc.T
        out[sl, :, 1] = scale.T
    return out


def run(inputs_dict, trace=False, trace_kwargs=None):
    in_maps, with_b1h, epsT = _host_prep(**inputs_dict)
    nc = _get_nc(with_b1h)
    res = run_bass_kernel_spmd(
        nc, in_maps, list(range(N_CORES)), trace=trace,
        **(trace_kwargs or {}),
    )
    _CACHE["last_results"] = res
    return _postprocess(res.results, epsT)


def kernel(**inputs):
    return run(inputs, trace=bool(os.environ.get("KERNEL_TRACE")))
